# revision 14
# baseline (speedup 1.0000x reference)
"""MoE audio projector kernel for 8 Trainium2 NeuronCores.

Strategy (expert-parallel, sparse dispatch):
  Host: depthwise conv + residual, fold K frames, RMSNorm, sigmoid router,
        top-2 + combine weights, per-expert token gather (all tiny FLOPs).
  Device (8 cores): core c handles expert c//2 with H-half c%2 over only the
        tokens routed to that expert, plus a 1/8 H-slice of the shared
        expert over all tokens. bf16 matmuls, fp32 PSUM accumulation.
  Host: sum shared partials, scatter-add expert partials.

Schedule design (from timeline-sim analysis; single-body sim 160us, unrolled
steady-state 147.4us vs a 146.1us tensor-engine floor):
  - Phase A (shared mm1) alone is DMA-bound: ntok (10.5MB) + w1sh (2.6MB)
    against 34us of PE. Fixes: (1) loads go on the SP queue in exact
    consumption order as [w1sh slice, nt ci=0 slice, nt ci=1 slice] per
    half/quarter k-group; (2) small tensors ride the Activation HWDGE
    queue; (3) expert mm1 for m=0 is interleaved into the phase-A k-group
    loop (lagging one group so its weight tile's DMA fits mid-stream),
    adding 8.8us of PE work that rebalances the phase.
  - The combine weights (esc >= 0) are folded into the nt->etok copy as a
    DVE bf16 multiply: relu is positively homogeneous and eb1 is zero, so
    pre-scaling expert tokens is exact (escb must be bf16 -- an fp32
    operand halves real-HW DVE rate and stalls nt tile recycling). Expert
    mm2 then needs no per-token output scaling, which lets it run
    "transposed" (tokens as the moving dim): 16 o-tiles x 8 k x cnt cycles
    instead of padded 128-row tiles -- 18% fewer PE cycles in that phase.
  - PSUM drains alternate DVE / Activation so neither engine gates PE.
  - In the timed reps-loop, iteration i+1's phase-A loads issue while
    iteration i's tail drains (no cross-iteration barrier), hiding the
    ~5us cold start; fp8 was evaluated and rejected (e4m3 everywhere gives
    5.3% rel err vs the 2e-2 gate; bf16 gives 0.4%).
"""

import math

import numpy as np
import ml_dtypes

import concourse.bass as bass
import concourse.bacc as bacc
import concourse.mybir as mybir
import concourse.tile as tile
from concourse.bass_utils import run_bass_kernel_spmd

BF16 = ml_dtypes.bfloat16
P = 128
B, S, D = 4, 1024, 1280
KF = 4                  # frames folded per token
IN = D * KF             # 5120
H = 2048
O = 2048
E = 4
TOPK = 2
TK = B * (S // KF)      # 1024 tokens
KT = IN // P            # 40 contraction tiles
KG = 8                  # k-tiles per DMA group
NKG = KT // KG          # 5 groups
H1E = H // 2            # expert H half per core
ME = H1E // P           # 8
H1S = H // 8            # shared H slice per core
MS = H1S // P           # 2
NO = O // 512           # 4 output col tiles (shared mm2)
OT = O // P             # 16 output row tiles (expert mm2, transposed)
EPS_RMS = 1e-8
EPS_W = 1e-6
NCORES = 8


def _chunks(total, step):
    """Split `total` into ceil(total/step) near-equal chunks (each <= step)."""
    n = (total + step - 1) // step
    base = total // n
    rem = total - base * n
    out = []
    off = 0
    for i in range(n):
        w = base + (1 if i < rem else 0)
        out.append((off, w))
        off += w
    return out


def host_preprocess(x, conv_w, conv_b, rms_w, router_w):
    """conv + fold + rmsnorm + router; returns (n [TK, IN] f32, combine [TK, E] f32)."""
    xp = np.pad(x, ((0, 0), (1, 1), (0, 0)))
    w0 = conv_w[:, 0, 0]
    w1 = conv_w[:, 0, 1]
    w2 = conv_w[:, 0, 2]
    xc = xp[:, :-2, :] * w0 + xp[:, 1:-1, :] * w1 + xp[:, 2:, :] * w2
    xr = x + xc + conv_b

    flat = xr.reshape(B, S // KF, IN).reshape(-1, IN)

    ms = np.mean(flat * flat, axis=-1, keepdims=True, dtype=np.float32)
    n = (flat * (1.0 / np.sqrt(ms + EPS_RMS)) * rms_w).astype(np.float32)

    logits = n @ router_w.T
    probs = 1.0 / (1.0 + np.exp(-logits))
    order = np.argsort(-probs, axis=1, kind="stable")
    idx = order[:, :TOPK]
    scores = np.take_along_axis(probs, idx, axis=1)
    w = scores / (scores.sum(axis=1, keepdims=True) + EPS_W)
    combine = np.zeros((n.shape[0], E), np.float32)
    rows = np.arange(n.shape[0])
    for j in range(TOPK):
        combine[rows, idx[:, j]] = w[:, j]
    return n, combine


def build_nc(TE, cnt=None, reps=1, escb_bf16=True, coarse_dma=False):
    """One SPMD program for all 8 cores.

    TE: padded per-expert token count (layout size, multiple of 128).
    cnt: actual max token count over experts (compute bound, <= TE).
    reps>1 wraps the body in a hardware loop (benchmark use only: repeats
    are idempotent; used for differential wall-clock timing).
    escb_bf16 / coarse_dma: experiment knobs (see bench scripts).
    """
    if cnt is None:
        cnt = TE
    dt = mybir.dt
    nc = bacc.Bacc()
    escdt = dt.bfloat16 if escb_bf16 else dt.float32

    resident_etok = TE <= 768   # pathological routing falls back to streaming

    ntok_d = nc.dram_tensor("ntok", [2, NKG, P, KG, 512], dt.bfloat16, kind="ExternalInput")
    ew1t_d = nc.dram_tensor("ew1t", [ME, P, KT, P], dt.bfloat16, kind="ExternalInput")
    ew2t_d = nc.dram_tensor("ew2t", [P, ME, O], dt.bfloat16, kind="ExternalInput")
    w1sh_d = nc.dram_tensor("w1sh", [P, KT, H1S], dt.bfloat16, kind="ExternalInput")
    w2sh_d = nc.dram_tensor("w2sh", [P, MS, O], dt.bfloat16, kind="ExternalInput")
    b1e_d = nc.dram_tensor("b1e", [P, ME], dt.float32, kind="ExternalInput")
    b1s_d = nc.dram_tensor("b1s", [P, MS], dt.float32, kind="ExternalInput")
    escb_d = nc.dram_tensor("escb", [P, TE], escdt, kind="ExternalInput")
    oute_d = nc.dram_tensor("oute", [O, TE], dt.bfloat16, kind="ExternalOutput")
    outs_d = nc.dram_tensor("outs", [TK, O], dt.bfloat16, kind="ExternalOutput")

    nch = _chunks(cnt, 512)      # token chunks for expert mm1 / mm2
    if not resident_etok:
        nch = [(0, 512), (512, cnt - 512)]
    relu = mybir.ActivationFunctionType.Relu
    copyf = mybir.ActivationFunctionType.Copy

    with tile.TileContext(nc) as tc:
        with (
            tc.tile_pool(name="res", bufs=1) as res,
            tc.tile_pool(name="wp", bufs=3) as wp,
            tc.tile_pool(name="npl", bufs=4) as npl,
            tc.tile_pool(name="opl", bufs=3) as opl,
            tc.tile_pool(name="psp", bufs=8, space="PSUM") as psp,
        ):

            def emit_body():
                ew2t = res.tile([P, ME, O], dt.bfloat16, name="ew2t")
                w1sh = res.tile([P, KT, H1S], dt.bfloat16, name="w1sh")
                w2sh = res.tile([P, MS, O], dt.bfloat16, name="w2sh")
                b1e = res.tile([P, ME], dt.float32, name="b1e")
                b1s = res.tile([P, MS], dt.float32, name="b1s")
                escb = res.tile([P, TE], escdt, name="escb")
                hte = res.tile([P, ME, TE], dt.bfloat16, name="hte")
                hts = res.tile([P, MS, TK], dt.bfloat16, name="hts")
                etok_res = None
                if resident_etok:
                    etok_res = res.tile([P, NKG, KG, TE], dt.bfloat16, name="etok")

                # ---- small tensors on the Activation HWDGE queue (no
                # bandwidth impact; keeps the SP queue pure).
                nc.scalar.dma_start(escb[:], escb_d[:])
                nc.scalar.dma_start(b1s[:], b1s_d[:])
                nc.scalar.dma_start(b1e[:], b1e_d[:])

                # ---- SP HWDGE queue, exact consumption order: per 2-ktile
                # quarter [w1sh slice, nt0 slice, nt1 slice]; the m=0 expert
                # weight tile drops in after group 1 (its matmuls lag one
                # group behind, see below).
                wt0 = None
                nt_pairs = []
                for g in range(NKG):
                    nt0 = npl.tile([P, KG, 512], dt.bfloat16, tag="ntok", name="nt0")
                    nt1 = npl.tile([P, KG, 512], dt.bfloat16, tag="ntok", name="nt1")
                    step = KG if coarse_dma else (2 if g == 0 else 4)
                    for j in range(0, KG, step):
                        js = slice(j, j + step)
                        ks = slice(g * KG + j, g * KG + j + step)
                        nc.sync.dma_start(w1sh[:, ks], w1sh_d[:, ks])
                        nc.sync.dma_start(nt0[:, js], ntok_d[0, g][:, js])
                        nc.sync.dma_start(nt1[:, js], ntok_d[1, g][:, js])
                        if g == 1 and j == 0 and resident_etok:
                            wt0 = wp.tile([P, KT, P], dt.bfloat16, tag="w1e", name="wt")
                            nc.sync.dma_start(wt0[:], ew1t_d[0])
                    nt_pairs.append((nt0, nt1))

                # ---- phase A compute: shared mm1 for both 512-token chunks
                # in one k-sweep (4 PSUM banks), esc folded into the
                # nt -> etok copy on DVE, expert mm1 m=0 interleaved lagging
                # one k-group behind the loads.
                pss = [
                    [
                        psp.tile([P, 512], dt.float32, tag="ps", name="ps_s1")
                        for _ in range(MS)
                    ]
                    for _ in range(2)
                ]
                pse0 = None
                if resident_etok:
                    pse0 = [
                        psp.tile([P, 512], dt.float32, tag="ps", name="ps_e1")[:, :w]
                        for (_, w) in nch
                    ]
                def emit_m0_group(g):
                    # expert mm1 m=0 over k-group g (etok filled; PSUM
                    # accumulation is additive so lagging k-order is fine --
                    # the start-flagged k=0 matmul still executes first).
                    for kk in range(KG):
                        k = g * KG + kk
                        for ci, (off, w) in enumerate(nch):
                            nc.tensor.matmul(
                                pse0[ci],
                                wt0[:, k],
                                etok_res[:, g, kk, off : off + w],
                                start=(k == 0),
                                stop=(k == KT - 1),
                            )

                for g in range(NKG):
                    nt_ci = nt_pairs[g]
                    for kk in range(KG):
                        k = g * KG + kk
                        for ci in range(2):
                            nt = nt_ci[ci]
                            cw = min(512, max(0, cnt - ci * 512))
                            if resident_etok and cw > 0:
                                nc.vector.tensor_mul(
                                    etok_res[:, g, kk, ci * 512 : ci * 512 + cw],
                                    nt[:, kk, :cw],
                                    escb[:, ci * 512 : ci * 512 + cw],
                                )
                            for m in range(MS):
                                nc.tensor.matmul(
                                    pss[ci][m],
                                    w1sh[:, k, m * P : (m + 1) * P],
                                    nt[:, kk],
                                    start=(k == 0),
                                    stop=(k == KT - 1),
                                )
                    if resident_etok and g >= 1:
                        emit_m0_group(g - 1)
                if resident_etok:
                    emit_m0_group(NKG - 1)
                for ci in range(2):
                    for m in range(MS):
                        nc.scalar.activation(
                            hts[:, m, ci * 512 : (ci + 1) * 512],
                            pss[ci][m],
                            relu,
                            bias=b1s[:, m : m + 1],
                            scale=1.0,
                        )
                if resident_etok:
                    for ci, (off, w) in enumerate(nch):
                        nc.scalar.activation(
                            hte[:, 0, off : off + w],
                            pse0[ci],
                            relu,
                            bias=b1e[:, 0:1],
                            scale=1.0,
                        )

                # ---- expert mm1 (m >= 1 when resident):
                # hte[:, m, :cnt] = relu(ew1t[m].T @ etok + b1e[m])
                # (etok is esc-pre-scaled; b1e is structurally zero here)
                m_start = 1 if resident_etok else 0
                for m in range(m_start, ME):
                    wt = wp.tile([P, KT, P], dt.bfloat16, tag="w1e", name="wt")
                    nc.sync.dma_start(wt[:], ew1t_d[m])
                    pse = [
                        psp.tile([P, 512], dt.float32, tag="ps", name="ps_e1")[:, :w]
                        for (_, w) in nch
                    ]
                    if resident_etok:
                        for k in range(KT):
                            for ci, (off, w) in enumerate(nch):
                                nc.tensor.matmul(
                                    pse[ci],
                                    wt[:, k],
                                    etok_res[:, k // KG, k % KG, off : off + w],
                                    start=(k == 0),
                                    stop=(k == KT - 1),
                                )
                    else:
                        # pathological routing (>768 tokens on one expert):
                        # re-stream 512-aligned token windows; slow but correct
                        for g in range(NKG):
                            nt2s = []
                            for ci, (off, w) in enumerate(nch):
                                nt2 = npl.tile(
                                    [P, KG, 512], dt.bfloat16, tag="ntok", name="nt2"
                                )
                                nc.sync.dma_start(nt2[:, :, :w], ntok_d[ci, g][:, :, :w])
                                for kk in range(KG):
                                    nc.vector.tensor_mul(
                                        nt2[:, kk, :w],
                                        nt2[:, kk, :w],
                                        escb[:, off : off + w],
                                    )
                                nt2s.append(nt2)
                            for kk in range(KG):
                                k = g * KG + kk
                                for ci, (off, w) in enumerate(nch):
                                    nc.tensor.matmul(
                                        pse[ci],
                                        wt[:, k],
                                        nt2s[ci][:, kk, :w],
                                        start=(k == 0),
                                        stop=(k == KT - 1),
                                    )
                    for ci, (off, w) in enumerate(nch):
                        nc.scalar.activation(
                            hte[:, m, off : off + w],
                            pse[ci],
                            relu,
                            bias=b1e[:, m : m + 1],
                            scale=1.0,
                        )

                nc.sync.dma_start(w2sh[:], w2sh_d[:])
                nc.sync.dma_start(ew2t[:], ew2t_d[:])

                # ---- shared mm2: outs rows = hts.T @ w2sh ----
                for t in range(TK // P):
                    pso = [
                        psp.tile([P, 512], dt.float32, tag="ps", name="ps_o")
                        for _ in range(NO)
                    ]
                    for k in range(MS):
                        for o in range(NO):
                            nc.tensor.matmul(
                                pso[o],
                                hts[:, k, t * P : (t + 1) * P],
                                w2sh[:, k, o * 512 : (o + 1) * 512],
                                start=(k == 0),
                                stop=(k == MS - 1),
                            )
                    ot = opl.tile([P, O], dt.bfloat16, tag="out", name="ot_s")
                    for o in range(NO):
                        dst = ot[:, o * 512 : (o + 1) * 512]
                        if o % 2 == 0:
                            nc.vector.tensor_copy(dst, pso[o])
                        else:
                            nc.scalar.activation(dst, pso[o], copyf, scale=1.0)
                    nc.sync.dma_start(outs_d[t * P : (t + 1) * P], ot[:])

                # ---- expert mm2 (transposed, tokens moving):
                # oute[o*128:(o+1)*128, :cnt] = (ew2t[:, :, o-tile].T @ hte)
                for o in range(OT):
                    ps2 = [
                        psp.tile([P, 512], dt.float32, tag="ps", name="ps_e2")[:, :w]
                        for (_, w) in nch
                    ]
                    for k in range(ME):
                        for ci, (off, w) in enumerate(nch):
                            nc.tensor.matmul(
                                ps2[ci],
                                ew2t[:, k, o * P : (o + 1) * P],
                                hte[:, k, off : off + w],
                                start=(k == 0),
                                stop=(k == ME - 1),
                            )
                    otT = opl.tile([P, 512 * len(nch)], dt.bfloat16, tag="outT", name="ot_e")
                    for ci, (off, w) in enumerate(nch):
                        dst = otT[:, off : off + w]
                        if ci % 2 == 0:
                            nc.vector.tensor_copy(dst, ps2[ci])
                        else:
                            nc.scalar.activation(dst, ps2[ci], copyf, scale=1.0)
                    nc.sync.dma_start(
                        oute_d[o * P : (o + 1) * P, :cnt], otT[:, :cnt]
                    )

            if reps == 1:
                emit_body()
            elif reps < 0:
                for _ in range(-reps):   # python-unrolled (sim only)
                    emit_body()
            else:
                with tc.For_i(0, reps, 1):
                    emit_body()

    nc.finalize()
    return nc


def _prepare(inputs):
    inp = {k: np.asarray(v, dtype=np.float32) for k, v in inputs.items()}
    n, combine = host_preprocess(
        inp["x"], inp["conv_w"], inp["conv_b"], inp["rms_w"], inp["router_w"]
    )
    nbf = n.astype(BF16)

    idxs = [np.nonzero(combine[:, e] > 0)[0] for e in range(E)]
    maxcnt = max(1, max(len(ix) for ix in idxs))
    TE = int(math.ceil(maxcnt / P) * P)

    all_tokens = np.arange(TK)
    perms = []
    in_maps = []
    for c in range(NCORES):
        e, hh = divmod(c, 2)
        sl = slice(hh * H1E, (hh + 1) * H1E)
        # ew1t[m, p, k, q] = W1h[m*128+q, k*128+p]  (lhsT layout, contiguous per (m,p))
        W1h = inp["ew1"][e, sl]                      # [H1E, IN]
        ew1t = np.ascontiguousarray(
            W1h.reshape(ME, P, KT, P).transpose(0, 3, 2, 1)
        ).astype(BF16)
        W2h = inp["ew2"][e][:, sl]                   # [O, H1E]
        ew2t = np.ascontiguousarray(
            W2h.T.reshape(ME, P, O).transpose(1, 0, 2)
        ).astype(BF16)
        ssl = slice(c * H1S, (c + 1) * H1S)
        w1sh = np.ascontiguousarray(
            inp["sw1"][ssl].T.reshape(KT, P, H1S).transpose(1, 0, 2)
        ).astype(BF16)
        w2sh = np.ascontiguousarray(
            inp["sw2"][:, ssl].T.reshape(MS, P, O).transpose(1, 0, 2)
        ).astype(BF16)
        b1e = np.ascontiguousarray(inp["eb1"][e, sl].reshape(ME, P).T).astype(np.float32)
        b1s = np.ascontiguousarray(inp["sb1"][ssl].reshape(MS, P).T).astype(np.float32)

        idx_e = idxs[e]
        cnt = len(idx_e)
        # permute tokens so this core's expert tokens come first; the expert
        # matmuls then reuse the prefix of the shared-expert token stream
        mask = np.zeros(TK, bool)
        mask[idx_e] = True
        perm = np.concatenate([idx_e, all_tokens[~mask]])
        perms.append(perm)
        ntok = np.ascontiguousarray(
            nbf[perm].T.reshape(NKG, KG, P, 2, 512).transpose(3, 0, 2, 1, 4)
        )
        esc = np.zeros((TE,), np.float32)
        esc[:cnt] = combine[idx_e, e]
        escb = np.ascontiguousarray(
            np.broadcast_to(esc[None, :], (P, TE)).astype(BF16)
        )

        in_maps.append(
            {
                "ntok": ntok,
                "ew1t": ew1t,
                "ew2t": ew2t,
                "w1sh": w1sh,
                "w2sh": w2sh,
                "b1e": b1e,
                "b1s": b1s,
                "escb": escb,
            }
        )
    return inp, combine, idxs, perms, TE, in_maps


def _assemble(inp, combine, idxs, perms, results):
    acc = np.zeros((TK, O), np.float32)
    for c in range(NCORES):
        acc[perms[c]] += results[c]["outs"].astype(np.float32)
    acc += inp["sb2"][None, :]
    acc += combine @ inp["eb2"]
    for c in range(NCORES):
        e = c // 2
        idx_e = idxs[e]
        cnt = len(idx_e)
        if cnt:
            acc[idx_e] += results[c]["oute"][:, :cnt].astype(np.float32).T
    return acc.reshape(B, S // KF, O)


def run(inputs, trace=False):
    inp, combine, idxs, perms, TE, in_maps = _prepare(inputs)
    maxcnt = max(1, max(len(ix) for ix in idxs))
    nc = build_nc(TE, cnt=maxcnt)
    res = run_bass_kernel_spmd(nc, in_maps, core_ids=list(range(NCORES)), trace=trace)
    out = _assemble(inp, combine, idxs, perms, res.results)
    return out, res


def kernel(**inputs):
    out, _ = run(inputs, trace=False)
    return out
